# revision 27
# baseline (speedup 1.0000x reference)
"""MoE FFN (8 experts, top-2) Trainium2 kernel.

Strategy (expert-pair x hidden-split, balanced):
  - Host computes the gate (softmax + top-2 + renormalize) in float64 and
    routes tokens per expert.
  - Experts are sorted by token count and paired big-with-small:
    pair p = (big_p, small_p). Cores 2p and 2p+1 both process ALL tokens of
    both experts in the pair, but each core handles one half of the hidden
    dim (2816 of 5632). Per-core weight traffic stays at one-expert
    equivalent (69 MB bf16) while the compute is balanced: per-core block
    sizes are S1 = pad8(max big count) and S2 = pad8(max small count),
    ~2080 columns vs 2128 for plain expert-parallel.
  - Each block runs the dense FFN for its expert / hidden half:
        H^T = silu(W1^T x^T) * (W3^T x^T)     (phase 1, H^T staged in DRAM)
        y^T = W2^T H^T                        (phase 2, partial over hidden)
    Host sums the two hidden halves and applies the gate weights.
  - PE order is p1(a) -> p1(b) -> p2(a) -> p2(b): phase-2 weight/H-strip
    DMAs for block a overlap phase-1 compute of block b, so the phase
    transition costs no PE idle.

DMA discipline (the thing that actually sets the pace):
  - Weights are host-swizzled so each streaming group is ONE dma_start with
    long per-partition runs (w13: 4 KB, w2: 22.5 KB) instead of 16-22
    short-row DMAs.
  - hbuf (H^T staging) is stored chunk-contiguous [128, HR*S]: each phase-2
    H-strip load is ONE contiguous dma_start.
  - Queue segregation: all LOADS go on the SP HWDGE ring (nc.sync) whose
    descriptors never wait on compute - deep prefetch; all STORES go on the
    ACT ring (nc.scalar); x loads use gpsimd (SWDGE). This avoids
    head-of-line blocking of prefetches behind store descriptors.

silu(g) is computed as g*sigmoid(g) (ACT sigmoid + DVE muls).
All matmul operands are bf16 (fp32 accumulation in PSUM).
"""

import math
from contextlib import ExitStack

import ml_dtypes
import numpy as np

P = 128
D_MODEL = 2048
HIDDEN = 5632
HALF_H = HIDDEN // 2    # hidden per core
HR = HALF_H // P        # h-rows per core (22)
DC = D_MODEL // P       # d chunks (16)
N_EXPERTS = 8
TOP_K = 2
N_CORES = 8

D_SPAN = 512            # phase-2 resident W2 span along d_model
DG = D_MODEL // D_SPAN  # 4
DTS = D_SPAN // P       # 4
CHUNK_MAX = 368         # n-chunk max (3 chunks, 7 PSUM banks in phase 1)

_prog_cache: dict[tuple, object] = {}


def _chunk_list(n_pad: int, max_chunk: int) -> list[tuple[int, int]]:
    """Split [0, n_pad) into near-equal chunks <= max_chunk, multiples of 8."""
    assert n_pad % 8 == 0
    k = math.ceil(n_pad / max_chunk)
    base = (n_pad // k) // 8 * 8
    sizes = [base] * k
    extra = n_pad - base * k
    i = 0
    while extra > 0:
        sizes[i] += 8
        extra -= 8
        i = (i + 1) % k
    out = []
    n0 = 0
    for s in sizes:
        out.append((n0, s))
        n0 += s
    return out



def _block_chunks(S: int) -> list[tuple[int, int]]:
    # S <= 1024 fits 2 PSUM-bank-wide chunks of <=512 (fewer matmuls)
    return _chunk_list(S, 512) if S <= 1024 else _chunk_list(S, CHUNK_MAX)

def _build_program(S1: int, S2: int):
    import concourse.bacc as bacc
    import concourse.mybir as mybir
    import concourse.tile as tile

    f32 = mybir.dt.float32
    bf16 = mybir.dt.bfloat16
    Sigmoid = mybir.ActivationFunctionType.Sigmoid
    mult = mybir.AluOpType.mult

    nc = bacc.Bacc(
        "TRN2",
        target_bir_lowering=False,
        debug=False,
        enable_asserts=False,
        num_devices=N_CORES,
    )
    # x^T per block, plain [d, S] layout (16 per-chunk gpsimd DMAs each)
    xa = nc.dram_tensor("xa", [D_MODEL, S1], bf16, kind="ExternalInput").ap()
    xb = nc.dram_tensor("xb", [D_MODEL, S2], bf16, kind="ExternalInput").ap()
    # host-swizzled weights:
    #   w1/w3: [128, HR*DC*128], group r (one h-row) contiguous 4 KB/partition
    #   w2:    [128, DG*HR*512], dgroup contiguous 22.5 KB/partition
    WSZ = HR * DC * P
    w1a = nc.dram_tensor("w1a", [P, WSZ], bf16, kind="ExternalInput").ap()
    w3a = nc.dram_tensor("w3a", [P, WSZ], bf16, kind="ExternalInput").ap()
    w1b = nc.dram_tensor("w1b", [P, WSZ], bf16, kind="ExternalInput").ap()
    w3b = nc.dram_tensor("w3b", [P, WSZ], bf16, kind="ExternalInput").ap()
    W2SZ = DG * HR * D_SPAN
    w2a = nc.dram_tensor("w2a", [P, W2SZ], bf16, kind="ExternalInput").ap()
    w2b = nc.dram_tensor("w2b", [P, W2SZ], bf16, kind="ExternalInput").ap()
    # H^T staging, chunk-contiguous: chunk (n0, sz) occupies
    # [:, HR*n0 : HR*n0 + HR*sz], h-row r at sub-offset r*sz.
    hbufa = nc.dram_tensor("hbufa", [P, HR * S1], bf16).ap()
    hbufb = nc.dram_tensor("hbufb", [P, HR * S2], bf16).ap()
    yT = nc.dram_tensor("yT", [D_MODEL, S1 + S2], f32, kind="ExternalOutput").ap()

    blocks = [
        ("a", S1, 0, xa, w1a, w3a, w2a, hbufa),
        ("b", S2, S1, xb, w1b, w3b, w2b, hbufb),
    ]

    with tile.TileContext(nc) as tc, ExitStack() as ctx:
        # Pools opened before the phase-1 pools: their SBUF ranges do not
        # overlap phase-1's, so phase-2 W2/H-strip DMAs overlap phase-1.
        w2pool = ctx.enter_context(tc.tile_pool(name="w2p", bufs=2))
        hinpool = ctx.enter_context(tc.tile_pool(name="hin", bufs=2))
        ypool = ctx.enter_context(tc.tile_pool(name="yst", bufs=3))

        # ---- phase 1: H^T = silu(W1^T x^T) * (W3^T x^T), streamed to DRAM
        with ExitStack() as p1:
            # resident x^T for both blocks, one tile per 128-row d-chunk;
            # loaded via gpsimd (SWDGE) so it doesn't queue behind weights
            xpool = p1.enter_context(tc.tile_pool(name="xp", bufs=1))
            xts = {}
            for bn, S, off, xT, _w1, _w3, _w2, _hb in blocks:
                for c in range(DC):
                    t = xpool.tile([P, S], bf16, tag=f"x{bn}{c}", name=f"x{bn}{c}")
                    xts[bn, c] = t
            # ALL x goes on the SP ring: chunk a0 now, a1-15 right after
            # row 0's weights, and block b's x sliced into the weight-row
            # stream from row 2 on (hooks in the loop below). The ring FIFO
            # delays xb past the startup window, so the startup xa stream
            # gets the full HBM bandwidth - gpsimd SWDGE DMAs would execute
            # at t=0 regardless of program position and round-robin packets
            # against xa, halving its rate (measured).
            nc.sync.dma_start(out=xts["a", 0][:], in_=xa[0:P, :])

            wpool = p1.enter_context(tc.tile_pool(name="w13", bufs=2))
            pspool = p1.enter_context(tc.tile_pool(name="ps1", bufs=1, space="PSUM"))
            spool = p1.enter_context(tc.tile_pool(name="sg", bufs=2))
            hpool = p1.enter_context(tc.tile_pool(name="hout", bufs=3))

            chunks1 = _block_chunks(S1)

            # PE warm-up: ~72 junk matmuls on a memset tile during the initial
            # DMA wait so HAM un-throttles (K=8/8) before the first real MM.
            # Lands in hbufa[:, 0:64]; real h-row 0 chunk 0 overwrites (WAW).
            wsrc = spool.tile([P, P], bf16, tag="warm_src", name="wsrc")
            nc.vector.memset(wsrc[:], 0.0)
            wps = pspool.tile(
                [P, chunks1[0][1]], f32, tag="pg0", bufs=2, name="warm_ps"
            )
            for i in range(72):
                nc.tensor.matmul(
                    wps[:, :64], wsrc[:], wsrc[:, :64],
                    start=(i == 0), stop=(i == 71),
                )
            wsb = spool.tile([P, 64], bf16, tag="warm_sb", name="wsb")
            nc.scalar.copy(wsb[:], wps[:, :64])
            nc.scalar.dma_start(out=hbufa[:, 0:64], in_=wsb[:])

            for bn, S, off, xT, w1, w3, _w2, hbuf in blocks:
                chunks = _block_chunks(S)
                for r in range(HR):
                    w1g = wpool.tile([P, DC * P], bf16, tag="w1g", name="w1g")
                    nc.sync.dma_start(
                        out=w1g[:], in_=w1[:, r * DC * P : (r + 1) * DC * P]
                    )
                    w3g = wpool.tile([P, DC * P], bf16, tag="w3g", name="w3g")
                    nc.sync.dma_start(
                        out=w3g[:], in_=w3[:, r * DC * P : (r + 1) * DC * P]
                    )
                    if bn == "a" and r == 0:
                        # rest of block a's x right behind row 0's weights
                        for c in range(1, DC):
                            nc.sync.dma_start(
                                out=xts["a", c][:], in_=xa[c * P : (c + 1) * P, :]
                            )
                    if bn == "a" and 2 <= r <= 16 and r % 2 == 0:
                        # two xb chunks per even row: fully landed by row 16
                        # (~240us), needed at ~312us; each slot adds ~1.5us
                        # to the ring per 14.8us row, so weight prefetch
                        # keeps its lookahead
                        for c in (r - 2, r - 1):
                            nc.sync.dma_start(
                                out=xts["b", c][:], in_=xb[c * P : (c + 1) * P, :]
                            )
                    htile = hpool.tile([P, S], bf16, tag="ht", name="ht")
                    pgs = [
                        pspool.tile(
                            [P, sz], f32, tag=f"pg{j}",
                            bufs=(2 if j == 0 else 1), name=f"pg{j}",
                        )
                        for j, (n0, sz) in enumerate(chunks)
                    ]
                    pvs = [
                        pspool.tile([P, sz], f32, tag=f"pv{j}", name=f"pv{j}")
                        for j, (n0, sz) in enumerate(chunks)
                    ]
                    for c in range(DC):
                        lhs = w1g[:, c * P : (c + 1) * P]
                        for j, (n0, sz) in enumerate(chunks):
                            nc.tensor.matmul(
                                pgs[j][:],
                                lhs,
                                xts[bn, c][:, n0 : n0 + sz],
                                start=(c == 0),
                                stop=(c == DC - 1),
                            )
                    for c in range(DC):
                        lhs = w3g[:, c * P : (c + 1) * P]
                        for j, (n0, sz) in enumerate(chunks):
                            nc.tensor.matmul(
                                pvs[j][:],
                                lhs,
                                xts[bn, c][:, n0 : n0 + sz],
                                start=(c == 0),
                                stop=(c == DC - 1),
                            )
                    for j, (n0, sz) in enumerate(chunks):
                        sg_t = spool.tile([P, sz], f32, tag="sg", name="sg_t")
                        nc.scalar.activation(sg_t[:], pgs[j][:], Sigmoid)
                        gv_t = spool.tile([P, sz], f32, tag="gv", name="gv_t")
                        nc.vector.tensor_tensor(gv_t[:], sg_t[:], pgs[j][:], op=mult)
                        nc.vector.tensor_tensor(
                            htile[:, n0 : n0 + sz], gv_t[:], pvs[j][:], op=mult
                        )
                        nc.scalar.dma_start(
                            out=hbuf[:, HR * n0 + r * sz : HR * n0 + (r + 1) * sz],
                            in_=htile[:, n0 : n0 + sz],
                        )

        # ---- phase 2: y^T += W2^T H^T (partial over this core's hidden half)
        with ExitStack() as p2:
            ps2 = p2.enter_context(tc.tile_pool(name="ps2", bufs=2, space="PSUM"))
            for bn, S, off, _xT, _w1, _w3, w2, hbuf in blocks:
                chunks = _block_chunks(S)
                for dg in range(DG):
                    d0 = dg * D_SPAN
                    w2g = w2pool.tile([P, HR * D_SPAN], bf16, tag="w2g", name="w2g")
                    nc.sync.dma_start(
                        out=w2g[:],
                        in_=w2[:, dg * HR * D_SPAN : (dg + 1) * HR * D_SPAN],
                    )
                    for n0, sz in chunks:
                        hstrip = hinpool.tile([P, HR * sz], bf16, tag="hs", name="hs")
                        nc.sync.dma_start(
                            out=hstrip[:],
                            in_=hbuf[:, HR * n0 : HR * n0 + HR * sz],
                        )
                        ps = [
                            ps2.tile([P, sz], f32, tag=f"yp{q}", name=f"yp{q}")
                            for q in range(DTS)
                        ]
                        for r in range(HR):
                            for q in range(DTS):
                                nc.tensor.matmul(
                                    ps[q][:],
                                    w2g[:, r * D_SPAN + q * P : r * D_SPAN + (q + 1) * P],
                                    hstrip[:, r * sz : (r + 1) * sz],
                                    start=(r == 0),
                                    stop=(r == HR - 1),
                                )
                        for q in range(DTS):
                            yst = ypool.tile([P, sz], f32, tag="yst", name="yst")
                            # alternate ACT/DVE so the final drain pipelines
                            if q % 2 == 0:
                                nc.scalar.copy(yst[:], ps[q][:])
                            else:
                                nc.vector.tensor_copy(yst[:], ps[q][:])
                            nc.scalar.dma_start(
                                out=yT[
                                    d0 + q * P : d0 + (q + 1) * P,
                                    off + n0 : off + n0 + sz,
                                ],
                                in_=yst[:],
                            )

    nc.compile()
    return nc


def _get_program(S1: int, S2: int):
    key = (S1, S2)
    if key not in _prog_cache:
        _prog_cache[key] = _build_program(S1, S2)
    return _prog_cache[key]


def _route(x2d: np.ndarray, Wg: np.ndarray):
    """Host gate: float64 softmax + top-2 + renormalize."""
    logits = x2d.astype(np.float64) @ Wg.astype(np.float64)
    logits -= logits.max(axis=-1, keepdims=True)
    e = np.exp(logits)
    p = e / e.sum(axis=-1, keepdims=True)
    top = np.argsort(-p, axis=-1, kind="stable")[:, :TOP_K]
    w = np.take_along_axis(p, top, axis=-1)
    w = w / w.sum(axis=-1, keepdims=True)
    return top, w.astype(np.float32)


def _pad8(n: int) -> int:
    return max(((n + 7) // 8) * 8, 24)


def _swizzle_w13(A: np.ndarray, h0: int) -> np.ndarray:
    """[2048, 5632] -> [128, HR*DC*128] with h-row-major groups:
    out[p, r*DC*128 + c*128 + j] = A[c*128+p, h0 + r*128 + j]."""
    B = A[:, h0 : h0 + HALF_H].reshape(DC, P, HR, P)
    return np.ascontiguousarray(B.transpose(1, 2, 0, 3).reshape(P, HR * DC * P))


def _swizzle_w2(A: np.ndarray, h0: int) -> np.ndarray:
    """[5632, 2048] -> [128, DG*HR*512] with dgroup-major:
    out[p, dg*HR*512 + r*512 + j] = A[h0 + r*128 + p, dg*512 + j]."""
    C = A[h0 : h0 + HALF_H].reshape(HR, P, DG, D_SPAN)
    return np.ascontiguousarray(C.transpose(1, 2, 0, 3).reshape(P, DG * HR * D_SPAN))


def _prepare(inputs: dict):
    x = np.asarray(inputs["x"], dtype=np.float32)
    Wg = np.asarray(inputs["Wg"], dtype=np.float32)
    W1 = np.asarray(inputs["W1"], dtype=np.float32)
    W3 = np.asarray(inputs["W3"], dtype=np.float32)
    W2 = np.asarray(inputs["W2"], dtype=np.float32)

    b, s, d = x.shape
    T = b * s
    x2d = np.ascontiguousarray(x.reshape(T, d))

    top, wts = _route(x2d, Wg)

    tok_lists = []
    wt_lists = []
    for e in range(N_EXPERTS):
        mask = top == e  # [T, K]
        toks = np.where(mask.any(axis=-1))[0]
        we = wts[toks][mask[toks]]  # one weight per selected token
        tok_lists.append(toks)
        wt_lists.append(we.astype(np.float32))

    counts = np.array([len(t) for t in tok_lists])
    order = np.argsort(-counts, kind="stable")  # big -> small
    bigs = order[:4]
    smalls = order[4:][::-1]  # pair biggest big with smallest small
    pairs = list(zip(bigs.tolist(), smalls.tolist()))

    S1 = _pad8(int(counts[bigs].max()))
    S2 = _pad8(int(counts[smalls].max()))

    nc = _get_program(S1, S2)

    W1bf = W1.astype(ml_dtypes.bfloat16)
    W3bf = W3.astype(ml_dtypes.bfloat16)
    W2bf = W2.astype(ml_dtypes.bfloat16)
    x2dbf = x2d.astype(ml_dtypes.bfloat16)

    in_maps = [None] * N_CORES
    for pi, (ea, eb) in enumerate(pairs):
        ta, tb = tok_lists[ea], tok_lists[eb]
        xae = np.zeros((d, S1), dtype=ml_dtypes.bfloat16)
        xae[:, : len(ta)] = x2dbf[ta].T
        xbe = np.zeros((d, S2), dtype=ml_dtypes.bfloat16)
        xbe[:, : len(tb)] = x2dbf[tb].T
        for half in range(2):
            h0 = half * HALF_H
            in_maps[2 * pi + half] = {
                "xa": xae,
                "xb": xbe,
                "w1a": _swizzle_w13(W1bf[ea], h0),
                "w3a": _swizzle_w13(W3bf[ea], h0),
                "w2a": _swizzle_w2(W2bf[ea], h0),
                "w1b": _swizzle_w13(W1bf[eb], h0),
                "w3b": _swizzle_w13(W3bf[eb], h0),
                "w2b": _swizzle_w2(W2bf[eb], h0),
            }

    plan = {
        "pairs": pairs,
        "S1": S1,
        "S2": S2,
        "tok_lists": tok_lists,
        "wt_lists": wt_lists,
    }
    return nc, in_maps, plan, None, (b, s, d)


def _combine(results, plan, _unused, shape):
    b, s, d = shape
    out2d = np.zeros((b * s, d), dtype=np.float32)
    S1 = plan["S1"]
    for pi, (ea, eb) in enumerate(plan["pairs"]):
        y0 = results[2 * pi]["yT"]
        y1 = results[2 * pi + 1]["yT"]
        for e, off in ((ea, 0), (eb, S1)):
            toks = plan["tok_lists"][e]
            n = len(toks)
            ye = (y0[:, off : off + n] + y1[:, off : off + n]).T  # [n_e, d]
            out2d[toks] += plan["wt_lists"][e][:, None] * ye
    return out2d.reshape(b, s, d)


def _ensure_trace_hooks():
    """If BASS_TRACE is set, run_bass_kernel_spmd imports antenv.axon_hooks,
    which some images lack. Provide the standard shim (ctypes into the axon
    .so) when missing, and make the artifact upload failure-tolerant."""
    import sys

    try:
        import antenv.axon_hooks  # noqa: F401
        return
    except ImportError:
        pass
    import contextlib
    import ctypes
    import types

    so_path = "/opt/axon/libaxon_pjrt.so"
    hook = None
    try:
        lib = ctypes.CDLL(so_path)
        lib.axon_start_nrt_profile.argtypes = [
            ctypes.POINTER(ctypes.c_int64),
            ctypes.c_size_t,
        ]
        lib.axon_start_nrt_profile.restype = ctypes.c_int64
        lib.axon_stop_nrt_profile.argtypes = [ctypes.c_char_p]
        lib.axon_stop_nrt_profile.restype = ctypes.c_int64

        @contextlib.contextmanager
        def _hook(output_dir, device_ids):
            import jax

            jax.devices()
            if device_ids:
                ids = (ctypes.c_int64 * len(device_ids))(*device_ids)
                rc = lib.axon_start_nrt_profile(ids, len(device_ids))
            else:
                rc = lib.axon_start_nrt_profile(None, 0)
            if rc != 0:
                raise RuntimeError(f"axon_start_nrt_profile rc={rc}")
            try:
                yield
            finally:
                lib.axon_stop_nrt_profile(str(output_dir).encode())

        hook = _hook
    except Exception:
        hook = None

    mod = types.ModuleType("antenv.axon_hooks")
    state = {"hook": hook}
    mod.get_axon_ntff_profile_hook = lambda: state["hook"]
    mod.set_axon_ntff_profile_hook = lambda h: state.update(hook=h)
    sys.modules["antenv.axon_hooks"] = mod
    try:
        import antenv

        antenv.axon_hooks = mod
    except ImportError:
        pass

    import concourse.bass_utils as bu

    orig_upload = bu.upload_artifacts

    def _safe_upload(tmpdir):
        try:
            return orig_upload(tmpdir)
        except Exception:
            return f"local://{tmpdir}"

    bu.upload_artifacts = _safe_upload


def kernel(**inputs) -> np.ndarray:
    from concourse.bass_utils import run_bass_kernel_spmd

    _ensure_trace_hooks()
    nc, in_maps, plan, _unused, shape = _prepare(inputs)
    res = run_bass_kernel_spmd(nc, in_maps, core_ids=list(range(N_CORES)))
    return _combine(res.results, plan, _unused, shape)
